# revision 49
# baseline (speedup 1.0000x reference)
"""Multi-head attention (B=4, T=2048, D=1024, H=16) on 8 Trainium2 NeuronCores.

Sharding: core = (batch, head-group): b = core // 2, g = core % 2.
Each core computes heads [g*8, g*8+8) of batch b:
  - Q/K projections into transposed layout qT/kT = W_g @ x_b.T  [512, 2048]
  - V projection in natural layout [2048, 512], emitted per head-pair
    (+ ones column per head; the V bias is folded into the post-normalize
    output copy instead)
  - scores computed transposed: S.T tile = K_h @ Q_h.T on the PE
  - exp fused on ScalarE over two-bank PSUM groups (FD=1024), scale=1/sqrt(64),
    no max subtraction (logits ~N(0,1))
  - PV in the flipped orientation: lhsT = P.T tile [tk,128tq] (stationary),
    rhs = [V_h | 1] [tk, 65] (moving) -> O natural [128tq, 65] in PSUM.
    This charges 65 free-columns per k-step instead of 512, halving the PE
    cost of PV; the softmax row-sums land in column 64.
  - normalize on DVE: reciprocal of col 64, then per-partition
    tensor_scalar_mul into an o_nat staging tile [tq, f] (bf16)
  - PE transpose (identity matmul) turns o_nat 128x128 tiles into O.T in
    PSUM; the DVE copy back to SBUF adds the V bias per partition
  - partial output projection yT_g = Wo[:, g].T-contraction, bf16 partials
Host: y[b] = (yT_part[2b] + yT_part[2b+1]).T + bo (f32 accumulate).

Pipelining: windows run pair-outer/chunk-inner. The scores+exp phase of a
window is ScalarE-paced (the two score PSUM slots recycle at exp speed), so
all other PE work is emitted as deferred fill behind it: the PV matmuls of
the PREVIOUS window (whose exps are complete, so they are always ready),
the o_nat transposes from two windows back, the next K m-tile / Q chunk /
V pair projections, and the trailing output projection. x is DMA'd
chunk-major so the first K/Q projections (and first exps) start as soon as
the first 1MB lands rather than after the full 4MB.

Self-contained: hardcodes all shapes; requires only concourse (bass) + numpy.
"""

import numpy as np

B, T, D = 4, 2048, 1024
H, HD = 16, 64
HG, DG = 8, 512          # heads / feature columns per core
NCORES = 8
P = 128
KD = D // P              # 8  k-tiles over model dim
MQ = DG // P             # 4  partition tiles of qT/kT/oT (one per head pair)
TK = T // P              # 16 key tiles
TQC = 512                # query-chunk (= one fp32 PSUM bank)
NC2 = T // TQC           # 4  query chunks
VW = HD + 1              # V columns per head incl. ones column
SCALE = 0.125            # 1/sqrt(HD)

_CACHE: dict = {}


def _emit(tc, aps, dbg=None, reps=1):
    import concourse.bass as bass  # noqa: F401
    from concourse import mybir

    nc = tc.nc
    dt = mybir.dt
    f32, bf16 = dt.float32, dt.bfloat16
    AF = mybir.ActivationFunctionType
    xT, wq, wk, wv, wo, bq, bk, bvT, ident, yT = (
        aps["xT"], aps["wq"], aps["wk"], aps["wv"], aps["wo"],
        aps["bq"], aps["bk"], aps["bvT"], aps["ident"], aps["yT"],
    )

    from contextlib import ExitStack

    with ExitStack() as ctx:
        const = ctx.enter_context(tc.tile_pool(name="const", bufs=1))
        persist = ctx.enter_context(tc.tile_pool(name="persist", bufs=1))
        xw = ctx.enter_context(tc.tile_pool(name="xw", bufs=1))
        ptp = ctx.enter_context(tc.tile_pool(name="ptp", bufs=4))
        onp = ctx.enter_context(tc.tile_pool(name="onp", bufs=2))
        yop = ctx.enter_context(tc.tile_pool(name="yop", bufs=3))
        nrm = ctx.enter_context(tc.tile_pool(name="nrm", bufs=2))
        scps = ctx.enter_context(tc.tile_pool(name="scps", bufs=2, space="PSUM"))
        qkvps = ctx.enter_context(tc.tile_pool(name="qkvps", bufs=2, space="PSUM"))
        pvps = ctx.enter_context(tc.tile_pool(name="pvps", bufs=2, space="PSUM"))

        # ---- persistent SBUF ----
        q_sb = persist.tile([P, MQ, T], bf16)
        k_sb = persist.tile([P, MQ, T], bf16)
        v_sb = persist.tile([P, TK, HG * VW], bf16)
        o_sb = persist.tile([P, MQ, T], bf16)
        v4d = v_sb.rearrange("p t (h c) -> p t h c", h=HG)
        nc.vector.memset(v4d[:, :, :, HD : HD + 1], 1.0)

        # ---- input DMAs. The DGE charges a fixed ~0.6us per DMA instruction
        # and the DMA engines serialize transfers, so the critical prefix is
        # kept tiny: wk/wq mt0 column slices (256KB each) + x chunk 0 unblock
        # the first scores; everything else streams behind them.
        x_sb = xw.tile([P, KD, T], bf16)
        wq_sb = xw.tile([P, KD, DG], bf16)
        wk_sb = xw.tile([P, KD, DG], bf16)
        wv_sb = xw.tile([P, KD, DG], bf16)
        # one queue: the DGE round-robins across queues, so a single queue is
        # the only way to pin the exact global transfer order (and it matches
        # the scheduler's serial-DMA assumption)
        nc.sync.dma_start(out=wk_sb[:, :, 0:P], in_=wk[:, :, 0:P])
        # x chunk 0 in halves: the first K accumulation starts after ~0.5MB
        nc.sync.dma_start(out=x_sb[:, 0 : KD // 2, 0:TQC], in_=xT[:, 0 : KD // 2, 0:TQC])
        nc.sync.dma_start(out=x_sb[:, KD // 2 :, 0:TQC], in_=xT[:, KD // 2 :, 0:TQC])
        nc.sync.dma_start(out=wq_sb[:, :, 0:P], in_=wq[:, :, 0:P])
        bq_sb = const.tile([P, MQ], f32)
        nc.sync.dma_start(out=bq_sb, in_=bq)
        bk_sb = const.tile([P, MQ], f32)
        nc.sync.dma_start(out=bk_sb, in_=bk)
        for n in range(1, NC2):
            nc.sync.dma_start(
                out=x_sb[:, :, n * TQC : (n + 1) * TQC],
                in_=xT[:, :, n * TQC : (n + 1) * TQC],
            )
        bvT_sb = const.tile([P, MQ], f32)
        nc.sync.dma_start(out=bvT_sb, in_=bvT)
        nc.sync.dma_start(out=wv_sb, in_=wv)
        nc.sync.dma_start(out=wk_sb[:, :, P:DG], in_=wk[:, :, P:DG])
        nc.sync.dma_start(out=wq_sb[:, :, P:DG], in_=wq[:, :, P:DG])
        ident_sb = const.tile([P, P], bf16)
        nc.sync.dma_start(out=ident_sb, in_=ident)
        wo_sb = const.tile([P, MQ, D], bf16)
        nc.sync.dma_start(out=wo_sb, in_=wo)

        # warm the PE p-state during the DMA wait: a chain of matmuls on a
        # memset tile keeps the engine continuously busy so the real
        # projections start at full clock instead of ramping through them
        wu_sb = xw.tile([P, TQC], bf16)
        nc.vector.memset(wu_sb, 1.0)
        for _ in range(17):
            wps = pvps.tile([P, TQC], f32, tag="pv", name="wps")
            nc.tensor.matmul(wps, wu_sb[:, 0:P], wu_sb, start=True, stop=True)

        def emit_qk_part(mt, n, which="kq"):
            """One T-chunk (n) of the q and/or k projection for m-tile mt."""
            sel = {
                "k": ((wk_sb, bk_sb, k_sb),),
                "q": ((wq_sb, bq_sb, q_sb),),
                "kq": ((wk_sb, bk_sb, k_sb), (wq_sb, bq_sb, q_sb)),
            }[which]
            for w_sb, b_col, dst in sel:
                ps = qkvps.tile([P, TQC], f32, tag="qkv", name="ps_qkv")
                for ki in range(KD):
                    nc.tensor.matmul(
                        ps,
                        w_sb[:, ki, mt * P : (mt + 1) * P],
                        x_sb[:, ki, n * TQC : (n + 1) * TQC],
                        start=(ki == 0),
                        stop=(ki == KD - 1),
                    )
                nc.vector.tensor_scalar_add(
                    dst[:, mt, n * TQC : (n + 1) * TQC], ps, b_col[:, mt : mt + 1]
                )

        def emit_v_part(pair, tg):
            """V projection for head-pair `pair`, t-tile group tg (4 tiles)."""
            fb = pair * 2 * HD
            ps = pvps.tile([P, 4, P], f32, tag="pv", name="ps_v")
            for s in range(4):
                t = tg * 4 + s
                for ki in range(KD):
                    nc.tensor.matmul(
                        ps[:, s, :],
                        x_sb[:, ki, t * P : (t + 1) * P],
                        wv_sb[:, ki, fb : fb + P],
                        start=(ki == 0),
                        stop=(ki == KD - 1),
                    )
            pv2 = ps.rearrange("p s (h c) -> p s h c", h=2)
            for s in range(4):
                t = tg * 4 + s
                nc.vector.tensor_copy(
                    v4d[:, t, 2 * pair : 2 * pair + 2, 0:HD], pv2[:, s, :, :]
                )

        def scores_exp_pair(p, c, pts, k_interleave=False):
            """Packed scores for heads (2p, 2p+1); exp over two-bank groups
            (FD = 2*TQC). k_interleave: first window only — emit each later
            K-projection part right before the first tkp that reads it, so a
            late x chunk stalls only the scores that truly need it."""
            tq0 = c * TQC
            for tkp in range(TK // 2):
                if k_interleave and tkp in (2, 4, 6):
                    emit_qk_part(0, tkp // 2, "k")
                scs = [
                    scps.tile([P, 2, TQC], f32, tag="sc", name="sc0"),
                    scps.tile([P, 2, TQC], f32, tag="sc", name="sc1"),
                ]
                for u in range(2):
                    tk = 2 * tkp + u
                    for i in range(2):
                        hb = i * HD
                        nc.tensor.matmul(
                            scs[i][:, u, :],
                            k_sb[hb : hb + HD, p, tk * P : (tk + 1) * P],
                            q_sb[hb : hb + HD, p, tq0 : tq0 + TQC],
                            start=True,
                            stop=True,
                        )
                for i in range(2):
                    nc.scalar.activation(
                        pts[i][:, 2 * tkp : 2 * tkp + 2, :], scs[i], AF.Exp, scale=SCALE
                    )

        pv_pend = []
        tp_pend = []

        def pv_flip(p, c, i, pt, o_nat):
            """Flipped PV for head h = 2p + i: O natural [tq, 65] per 128-row
            tq subtile; the stationary is the P.T tile so each k-step streams
            only 65 V columns. Row-sums land in column 64; normalize on DVE
            into o_nat [tq, pair-feature]."""
            h = 2 * p + i
            opv = pvps.tile([P, MQ, VW], f32, tag="pv", name="opv")
            for s in range(MQ):
                for tk in range(TK):
                    nc.tensor.matmul(
                        opv[:, s, :],
                        pt[:, tk, s * P : (s + 1) * P],
                        v_sb[:, tk, h * VW : (h + 1) * VW],
                        start=(tk == 0),
                        stop=(tk == TK - 1),
                    )
            rc = nrm.tile([P, MQ, 1], f32, name="rc")
            nc.vector.reciprocal(rc, opv[:, :, HD : HD + 1])
            for s in range(MQ):
                nc.vector.tensor_scalar_mul(
                    o_nat[:, s, i * HD : (i + 1) * HD],
                    opv[:, s, 0:HD],
                    rc[:, s, :],
                )

        def flush_pv():
            """PV + normalize for the previous window: its exps are complete,
            so every matmul is immediately ready to fill ScalarE-paced gaps
            in the current window's scores phase."""
            while pv_pend:
                p, c, pts = pv_pend.pop(0)
                o_nat = onp.tile([P, MQ, P], bf16, tag="onat", name="o_nat")
                pv_flip(p, c, 0, pts[0], o_nat)
                pv_flip(p, c, 1, pts[1], o_nat)
                tp_pend.append((p, c, o_nat))

        def flush_tp():
            """PE-transpose o_nat 128x128 tiles into O.T (PSUM), then DVE-copy
            to o_sb adding the V bias per partition. Trails the PV by one
            more window so the PE never waits on the normalize."""
            while tp_pend:
                p, c, o_nat = tp_pend.pop(0)
                for s in range(MQ):
                    tps = pvps.tile([P, P], bf16, tag="pv", name="tps")
                    nc.tensor.transpose(tps, o_nat[:, s, :], ident_sb)
                    nc.vector.tensor_scalar_add(
                        o_sb[:, p, c * TQC + s * P : c * TQC + (s + 1) * P],
                        tps,
                        bvT_sb[:, p : p + 1],
                    )

        def emit_oproj(c):
            tq0 = c * TQC
            for j in range(D // P):
                ys = qkvps.tile([P, TQC], f32, tag="qkv", name="ys")
                for ki in range(MQ):
                    nc.tensor.matmul(
                        ys,
                        wo_sb[:, ki, j * P : (j + 1) * P],
                        o_sb[:, ki, tq0 : tq0 + TQC],
                        start=(ki == 0),
                        stop=(ki == MQ - 1),
                    )
                yo = yop.tile([P, TQC], bf16, name="yo")
                nc.vector.tensor_copy(yo, ys)
                nc.sync.dma_start(out=yT[:, j, tq0 : tq0 + TQC], in_=yo)

        def emit_tail_pv(p, c, pts, tp_pool=None, tp_tag="sc"):
            """s-pipelined PV -> normalize -> transpose chain emitted inside
            its own window (block-3 windows), so the PV matmuls dribble in as
            the exps land and the chunk completes a window earlier than the
            deferred path; the transposes borrow another pool's psum slots to
            avoid a rotation conflict with the still-live opv tiles."""
            tq0 = c * TQC
            o_nat = onp.tile([P, MQ, P], bf16, tag="onat", name="o_nat")
            opvs = [
                pvps.tile([P, MQ, VW], f32, tag="pv", name="opv"),
                pvps.tile([P, MQ, VW], f32, tag="pv", name="opv"),
            ]
            rcs = [
                nrm.tile([P, MQ, 1], f32, name="rc"),
                nrm.tile([P, MQ, 1], f32, name="rc"),
            ]
            for s in range(MQ):
                for i in range(2):
                    h = 2 * p + i
                    for tk in range(TK):
                        nc.tensor.matmul(
                            opvs[i][:, s, :],
                            pts[i][:, tk, s * P : (s + 1) * P],
                            v_sb[:, tk, h * VW : (h + 1) * VW],
                            start=(tk == 0),
                            stop=(tk == TK - 1),
                        )
                    nc.vector.reciprocal(rcs[i][:, s, :], opvs[i][:, s, HD : HD + 1])
                    nc.vector.tensor_scalar_mul(
                        o_nat[:, s, i * HD : (i + 1) * HD],
                        opvs[i][:, s, 0:HD],
                        rcs[i][:, s, :],
                    )
                tps = (tp_pool or scps).tile([P, P], bf16, tag=tp_tag, name="tps")
                nc.tensor.transpose(tps, o_nat[:, s, :], ident_sb)
                nc.vector.tensor_scalar_add(
                    o_sb[:, p, tq0 + s * P : tq0 + (s + 1) * P],
                    tps,
                    bvT_sb[:, p : p + 1],
                )

        def emit_tail_oproj():
            """Final chunk's output projection, s-split so each j consumes
            the 128-query subtiles as they land in o_sb."""
            tq0 = (NC2 - 1) * TQC
            for j in range(D // P):
                # alternate psum pools (pv/sc are both free in the tail;
                # qkv is left to the chunk-2 projection): 4 effective slots
                # so the yo copies pipeline instead of gating every other j
                if j % 2 == 0:
                    ys = pvps.tile([P, TQC], f32, tag="pv", name="ys")
                else:
                    ys = scps.tile([P, TQC], f32, tag="sc", name="ys")
                for s in range(MQ):
                    for ki in range(MQ):
                        nc.tensor.matmul(
                            ys[:, s * P : (s + 1) * P],
                            wo_sb[:, ki, j * P : (j + 1) * P],
                            o_sb[:, ki, tq0 + s * P : tq0 + (s + 1) * P],
                            start=(ki == 0),
                            stop=(ki == MQ - 1),
                        )
                yo = yop.tile([P, TQC], bf16, name="yo")
                if j % 2 == 0:
                    nc.vector.tensor_copy(yo, ys)
                else:
                    nc.scalar.activation(yo, ys, AF.Copy)
                nc.sync.dma_start(out=yT[:, j, tq0 : tq0 + TQC], in_=yo)

        # ---- schedule: pair-outer, chunk-inner ----
        if reps > 1:
            loop_cm = tc.For_i(0, reps, 1)
            loop_cm.__enter__()

        # only the first projections (K n0, Q(0,0)) precede the first scores
        # in emission: everything DMA-gated after that is emitted as window
        # fill so a late x chunk can never head-of-line block ready scores
        emit_qk_part(0, 0, "k")
        emit_qk_part(0, 0, "q")

        for p in range(MQ):
            for c in range(NC2):
                pts = [
                    ptp.tile([P, TK, TQC], bf16, tag="pt", name="pt0"),
                    ptp.tile([P, TK, TQC], bf16, tag="pt", name="pt1"),
                ]
                # max priority: a ready score matmul must always beat fill
                # work in the scheduler, else the exp stream (the pacer)
                # starves behind bulk projection work
                with tc.high_priority():
                    scores_exp_pair(p, c, pts, k_interleave=(p == 0 and c == 0))
                flush_tp()
                flush_pv()
                last2 = p == MQ - 2 and c == NC2 - 1  # window (2,3)
                if p == 0 and c < NC2 - 1:
                    emit_qk_part(0, c + 1, "q")
                if p == 0 and c == 0:
                    for tg in range(4):      # pair 0's V: all needed by (0,1)
                        emit_v_part(0, tg)
                if p < MQ - 1:
                    if not last2:
                        emit_v_part(p + 1, c)  # next pair's V, a block ahead
                    emit_qk_part(p + 1, c, "k" if last2 else "kq")
                if p == MQ - 1 and c == 0:
                    # fill shifted out of the crowded (2,3) window
                    emit_v_part(MQ - 1, NC2 - 1)
                    emit_qk_part(MQ - 1, NC2 - 1, "q")
                if p == MQ - 1 and c >= 1:
                    # shorten the transpose trail to one window so chunk c-1
                    # completes here, then project it as window fill
                    flush_tp()
                    if c == NC2 - 1:
                        # chunk 3's PV chain gates the end: ahead of oproj(2)
                        emit_tail_pv(p, c, pts)
                    emit_oproj(c - 1)
                if p == MQ - 1 and c == NC2 - 2:
                    # (3,2) runs its PV in-window too (transposes on the qkv
                    # slots — sc is busy with the next window's scores), so
                    # chunk 2 completes here and projects as (3,3) fill
                    emit_tail_pv(p, c, pts, tp_pool=qkvps, tp_tag="qkv")
                if not (p == MQ - 1 and c >= NC2 - 2):
                    pv_pend.append((p, c, pts))
        emit_tail_oproj()

        if reps > 1:
            loop_cm.__exit__(None, None, None)

        if reps > 1:
            loop_cm.__exit__(None, None, None)

        if dbg is not None:
            nc.sync.dma_start(out=dbg["q"], in_=q_sb)
            nc.sync.dma_start(out=dbg["k"], in_=k_sb)
            nc.sync.dma_start(out=dbg["v"], in_=v_sb)
            nc.sync.dma_start(out=dbg["o"], in_=o_sb)


def _build(debug=False, reps=1):
    import concourse.tile as tile
    from concourse import bacc, mybir

    dt = mybir.dt
    f32, bf16 = dt.float32, dt.bfloat16

    nc = bacc.Bacc("TRN2", target_bir_lowering=False, debug=False)
    # inputs are host-preswizzled into partition-major layouts so every DMA
    # descriptor is a fat contiguous run
    aps = {
        "xT": nc.dram_tensor("xT", [P, KD, T], bf16, kind="ExternalInput").ap(),
        "wq": nc.dram_tensor("wq", [P, KD, DG], bf16, kind="ExternalInput").ap(),
        "wk": nc.dram_tensor("wk", [P, KD, DG], bf16, kind="ExternalInput").ap(),
        "wv": nc.dram_tensor("wv", [P, KD, DG], bf16, kind="ExternalInput").ap(),
        "wo": nc.dram_tensor("wo", [P, MQ, D], bf16, kind="ExternalInput").ap(),
        "bq": nc.dram_tensor("bq", [P, MQ], f32, kind="ExternalInput").ap(),
        "bk": nc.dram_tensor("bk", [P, MQ], f32, kind="ExternalInput").ap(),
        "bvT": nc.dram_tensor("bvT", [P, MQ], f32, kind="ExternalInput").ap(),
        "ident": nc.dram_tensor("ident", [P, P], bf16, kind="ExternalInput").ap(),
        "yT": nc.dram_tensor("yT", [P, D // P, T], bf16, kind="ExternalOutput").ap(),
    }

    dbg = None
    if debug:
        dbg = {
            "q": nc.dram_tensor("dbg_q", [P, MQ, T], bf16, kind="ExternalOutput").ap(),
            "k": nc.dram_tensor("dbg_k", [P, MQ, T], bf16, kind="ExternalOutput").ap(),
            "v": nc.dram_tensor(
                "dbg_v", [P, TK, HG * VW], bf16, kind="ExternalOutput"
            ).ap(),
            "o": nc.dram_tensor("dbg_o", [P, MQ, T], bf16, kind="ExternalOutput").ap(),
        }

    with tile.TileContext(nc) as tc:
        _emit(tc, aps, dbg, reps=reps)
    nc.compile()
    return nc


def _get_nc():
    if "nc" not in _CACHE:
        _CACHE["nc"] = _build()
    return _CACHE["nc"]


def _shard_inputs(x, Wq, bq, Wk, bk, Wv, bv, Wo, bo):
    import ml_dtypes

    bf16 = ml_dtypes.bfloat16
    f32 = np.float32

    def c(a, dtype):
        return np.ascontiguousarray(a).astype(dtype)

    def kp(a, kt):  # [kt*P, F] -> [P, kt, F] partition-major swizzle
        return a.reshape(kt, P, a.shape[-1]).transpose(1, 0, 2)

    ident = np.eye(P, dtype=bf16)
    in_maps = []
    for core in range(NCORES):
        b, g = core // 2, core % 2
        hs = g * DG
        in_maps.append(
            {
                "xT": c(kp(x[b].T, KD), bf16),
                "wq": c(kp(Wq[hs : hs + DG, :].T, KD), bf16),
                "wk": c(kp(Wk[hs : hs + DG, :].T, KD), bf16),
                "wv": c(kp(Wv[hs : hs + DG, :].T, KD), bf16),
                "wo": c(kp(Wo[:, hs : hs + DG].T, MQ), bf16),
                "bq": c(bq[hs : hs + DG].reshape(MQ, P).T, f32),
                "bk": c(bk[hs : hs + DG].reshape(MQ, P).T, f32),
                "bvT": c(bv[hs : hs + DG].reshape(MQ, P).T, f32),
                "ident": ident,
            }
        )
    return in_maps


def _run(inputs, trace=False):
    from concourse import bass_utils

    nc = _get_nc()
    np_in = {k: np.asarray(v) for k, v in inputs.items()}
    in_maps = _shard_inputs(**np_in)
    res = bass_utils.run_bass_kernel_spmd(
        nc, in_maps, core_ids=list(range(NCORES)), trace=trace
    )
    bo = np_in["bo"].astype(np.float32)
    y = np.empty((B, T, D), dtype=np.float32)
    for b in range(B):
        acc = res.results[2 * b]["yT"].astype(np.float32) + res.results[
            2 * b + 1
        ]["yT"].astype(np.float32)  # [P, D/P, T]
        y[b] = acc.transpose(1, 0, 2).reshape(D, T).T + bo
    return y, res


def kernel(**inputs):
    y, _ = _run(inputs)
    return y


# revision 51
# speedup vs baseline: 1.0037x; 1.0037x over previous
"""Multi-head attention (B=4, T=2048, D=1024, H=16) on 8 Trainium2 NeuronCores.

Sharding: core = (batch, head-group): b = core // 2, g = core % 2.
Each core computes heads [g*8, g*8+8) of batch b:
  - Q/K projections into transposed layout qT/kT = W_g @ x_b.T  [512, 2048]
  - V projection in natural layout [2048, 512], emitted per head-pair
    (+ ones column per head; the V bias is folded into the post-normalize
    output copy instead)
  - scores computed transposed: S.T tile = K_h @ Q_h.T on the PE
  - exp fused on ScalarE over two-bank PSUM groups (FD=1024), scale=1/sqrt(64),
    no max subtraction (logits ~N(0,1))
  - PV in the flipped orientation: lhsT = P.T tile [tk,128tq] (stationary),
    rhs = [V_h | 1] [tk, 65] (moving) -> O natural [128tq, 65] in PSUM.
    This charges 65 free-columns per k-step instead of 512, halving the PE
    cost of PV; the softmax row-sums land in column 64.
  - normalize on DVE: reciprocal of col 64, then per-partition
    tensor_scalar_mul into an o_nat staging tile [tq, f] (bf16)
  - PE transpose (identity matmul) turns o_nat 128x128 tiles into O.T in
    PSUM; the DVE copy back to SBUF adds the V bias per partition
  - partial output projection yT_g = Wo[:, g].T-contraction, bf16 partials
Host: y[b] = (yT_part[2b] + yT_part[2b+1]).T + bo (f32 accumulate).

Pipelining: windows run pair-outer/chunk-inner. The scores+exp phase of a
window is ScalarE-paced (the two score PSUM slots recycle at exp speed), so
all other PE work is emitted as deferred fill behind it: the PV matmuls of
the PREVIOUS window (whose exps are complete, so they are always ready),
the o_nat transposes from two windows back, the next K m-tile / Q chunk /
V pair projections, and the trailing output projection. x is DMA'd
chunk-major so the first K/Q projections (and first exps) start as soon as
the first 1MB lands rather than after the full 4MB.

Self-contained: hardcodes all shapes; requires only concourse (bass) + numpy.
"""

import numpy as np

B, T, D = 4, 2048, 1024
H, HD = 16, 64
HG, DG = 8, 512          # heads / feature columns per core
NCORES = 8
P = 128
KD = D // P              # 8  k-tiles over model dim
MQ = DG // P             # 4  partition tiles of qT/kT/oT (one per head pair)
TK = T // P              # 16 key tiles
TQC = 512                # query-chunk (= one fp32 PSUM bank)
NC2 = T // TQC           # 4  query chunks
VW = HD + 1              # V columns per head incl. ones column
SCALE = 0.125            # 1/sqrt(HD)

_CACHE: dict = {}


def _emit(tc, aps, dbg=None, reps=1):
    import concourse.bass as bass  # noqa: F401
    from concourse import mybir

    nc = tc.nc
    dt = mybir.dt
    f32, bf16 = dt.float32, dt.bfloat16
    AF = mybir.ActivationFunctionType
    xT, wq, wk, wv, wo, bq, bk, bvT, ident, yT = (
        aps["xT"], aps["wq"], aps["wk"], aps["wv"], aps["wo"],
        aps["bq"], aps["bk"], aps["bvT"], aps["ident"], aps["yT"],
    )

    from contextlib import ExitStack

    with ExitStack() as ctx:
        const = ctx.enter_context(tc.tile_pool(name="const", bufs=1))
        persist = ctx.enter_context(tc.tile_pool(name="persist", bufs=1))
        xw = ctx.enter_context(tc.tile_pool(name="xw", bufs=1))
        ptp = ctx.enter_context(tc.tile_pool(name="ptp", bufs=4))
        onp = ctx.enter_context(tc.tile_pool(name="onp", bufs=2))
        yop = ctx.enter_context(tc.tile_pool(name="yop", bufs=3))
        nrm = ctx.enter_context(tc.tile_pool(name="nrm", bufs=2))
        scps = ctx.enter_context(tc.tile_pool(name="scps", bufs=2, space="PSUM"))
        qkvps = ctx.enter_context(tc.tile_pool(name="qkvps", bufs=2, space="PSUM"))
        pvps = ctx.enter_context(tc.tile_pool(name="pvps", bufs=2, space="PSUM"))

        # ---- persistent SBUF ----
        q_sb = persist.tile([P, MQ, T], bf16)
        k_sb = persist.tile([P, MQ, T], bf16)
        v_sb = persist.tile([P, TK, HG * VW], bf16)
        o_sb = persist.tile([P, MQ, T], bf16)
        v4d = v_sb.rearrange("p t (h c) -> p t h c", h=HG)
        nc.vector.memset(v4d[:, :, :, HD : HD + 1], 1.0)

        # ---- input DMAs. The DGE charges a fixed ~0.6us per DMA instruction
        # and the DMA engines serialize transfers, so the critical prefix is
        # kept tiny: wk/wq mt0 column slices (256KB each) + x chunk 0 unblock
        # the first scores; everything else streams behind them.
        x_sb = xw.tile([P, KD, T], bf16)
        wq_sb = xw.tile([P, KD, DG], bf16)
        wk_sb = xw.tile([P, KD, DG], bf16)
        wv_sb = xw.tile([P, KD, DG], bf16)
        # one queue: the DGE round-robins across queues, so a single queue is
        # the only way to pin the exact global transfer order (and it matches
        # the scheduler's serial-DMA assumption)
        nc.sync.dma_start(out=wk_sb[:, :, 0:P], in_=wk[:, :, 0:P])
        # x chunk 0 in halves: the first K accumulation starts after ~0.5MB
        nc.sync.dma_start(out=x_sb[:, 0 : KD // 2, 0:TQC], in_=xT[:, 0 : KD // 2, 0:TQC])
        nc.sync.dma_start(out=x_sb[:, KD // 2 :, 0:TQC], in_=xT[:, KD // 2 :, 0:TQC])
        nc.sync.dma_start(out=wq_sb[:, :, 0:P], in_=wq[:, :, 0:P])
        bq_sb = const.tile([P, MQ], f32)
        nc.sync.dma_start(out=bq_sb, in_=bq)
        bk_sb = const.tile([P, MQ], f32)
        nc.sync.dma_start(out=bk_sb, in_=bk)
        for n in range(1, NC2):
            nc.sync.dma_start(
                out=x_sb[:, :, n * TQC : (n + 1) * TQC],
                in_=xT[:, :, n * TQC : (n + 1) * TQC],
            )
        bvT_sb = const.tile([P, MQ], f32)
        nc.sync.dma_start(out=bvT_sb, in_=bvT)
        nc.sync.dma_start(out=wv_sb, in_=wv)
        nc.sync.dma_start(out=wk_sb[:, :, P:DG], in_=wk[:, :, P:DG])
        nc.sync.dma_start(out=wq_sb[:, :, P:DG], in_=wq[:, :, P:DG])
        ident_sb = const.tile([P, P], bf16)
        nc.sync.dma_start(out=ident_sb, in_=ident)
        wo_sb = const.tile([P, MQ, D], bf16)
        nc.sync.dma_start(out=wo_sb, in_=wo)

        # warm the PE p-state during the DMA wait: a chain of matmuls on a
        # memset tile keeps the engine continuously busy so the real
        # projections start at full clock instead of ramping through them
        wu_sb = xw.tile([P, TQC], bf16)
        nc.vector.memset(wu_sb, 1.0)
        for _ in range(17):
            wps = pvps.tile([P, TQC], f32, tag="pv", name="wps")
            nc.tensor.matmul(wps, wu_sb[:, 0:P], wu_sb, start=True, stop=True)

        def emit_qk_part(mt, n, which="kq"):
            """One T-chunk (n) of the q and/or k projection for m-tile mt."""
            sel = {
                "k": ((wk_sb, bk_sb, k_sb),),
                "q": ((wq_sb, bq_sb, q_sb),),
                "kq": ((wk_sb, bk_sb, k_sb), (wq_sb, bq_sb, q_sb)),
            }[which]
            for w_sb, b_col, dst in sel:
                ps = qkvps.tile([P, TQC], f32, tag="qkv", name="ps_qkv")
                for ki in range(KD):
                    nc.tensor.matmul(
                        ps,
                        w_sb[:, ki, mt * P : (mt + 1) * P],
                        x_sb[:, ki, n * TQC : (n + 1) * TQC],
                        start=(ki == 0),
                        stop=(ki == KD - 1),
                    )
                nc.vector.tensor_scalar_add(
                    dst[:, mt, n * TQC : (n + 1) * TQC], ps, b_col[:, mt : mt + 1]
                )

        def emit_v_part(pair, tg):
            """V projection for head-pair `pair`, t-tile group tg (4 tiles)."""
            fb = pair * 2 * HD
            ps = pvps.tile([P, 4, P], f32, tag="pv", name="ps_v")
            for s in range(4):
                t = tg * 4 + s
                for ki in range(KD):
                    nc.tensor.matmul(
                        ps[:, s, :],
                        x_sb[:, ki, t * P : (t + 1) * P],
                        wv_sb[:, ki, fb : fb + P],
                        start=(ki == 0),
                        stop=(ki == KD - 1),
                    )
            pv2 = ps.rearrange("p s (h c) -> p s h c", h=2)
            for s in range(4):
                t = tg * 4 + s
                nc.vector.tensor_copy(
                    v4d[:, t, 2 * pair : 2 * pair + 2, 0:HD], pv2[:, s, :, :]
                )

        def scores_exp_pair(p, c, pts, k_interleave=False):
            """Packed scores for heads (2p, 2p+1); exp over two-bank groups
            (FD = 2*TQC). k_interleave: first window only — emit each later
            K-projection part right before the first tkp that reads it, so a
            late x chunk stalls only the scores that truly need it."""
            tq0 = c * TQC
            for tkp in range(TK // 2):
                if k_interleave and tkp in (2, 4, 6):
                    emit_qk_part(0, tkp // 2, "k")
                scs = [
                    scps.tile([P, 2, TQC], f32, tag="sc", name="sc0"),
                    scps.tile([P, 2, TQC], f32, tag="sc", name="sc1"),
                ]
                for u in range(2):
                    tk = 2 * tkp + u
                    for i in range(2):
                        hb = i * HD
                        nc.tensor.matmul(
                            scs[i][:, u, :],
                            k_sb[hb : hb + HD, p, tk * P : (tk + 1) * P],
                            q_sb[hb : hb + HD, p, tq0 : tq0 + TQC],
                            start=True,
                            stop=True,
                        )
                for i in range(2):
                    nc.scalar.activation(
                        pts[i][:, 2 * tkp : 2 * tkp + 2, :], scs[i], AF.Exp, scale=SCALE
                    )

        pv_pend = []
        tp_pend = []

        def pv_flip(p, c, i, pt, o_nat):
            """Flipped PV for head h = 2p + i: O natural [tq, 65] per 128-row
            tq subtile; the stationary is the P.T tile so each k-step streams
            only 65 V columns. Row-sums land in column 64; normalize on DVE
            into o_nat [tq, pair-feature]."""
            h = 2 * p + i
            opv = pvps.tile([P, MQ, VW], f32, tag="pv", name="opv")
            for s in range(MQ):
                for tk in range(TK):
                    nc.tensor.matmul(
                        opv[:, s, :],
                        pt[:, tk, s * P : (s + 1) * P],
                        v_sb[:, tk, h * VW : (h + 1) * VW],
                        start=(tk == 0),
                        stop=(tk == TK - 1),
                    )
            rc = nrm.tile([P, MQ, 1], f32, name="rc")
            nc.vector.reciprocal(rc, opv[:, :, HD : HD + 1])
            for s in range(MQ):
                nc.vector.tensor_scalar_mul(
                    o_nat[:, s, i * HD : (i + 1) * HD],
                    opv[:, s, 0:HD],
                    rc[:, s, :],
                )

        def flush_pv():
            """PV + normalize for the previous window: its exps are complete,
            so every matmul is immediately ready to fill ScalarE-paced gaps
            in the current window's scores phase."""
            while pv_pend:
                p, c, pts = pv_pend.pop(0)
                o_nat = onp.tile([P, MQ, P], bf16, tag="onat", name="o_nat")
                pv_flip(p, c, 0, pts[0], o_nat)
                pv_flip(p, c, 1, pts[1], o_nat)
                tp_pend.append((p, c, o_nat))

        def flush_tp():
            """PE-transpose o_nat 128x128 tiles into O.T (PSUM), then DVE-copy
            to o_sb adding the V bias per partition. Trails the PV by one
            more window so the PE never waits on the normalize."""
            while tp_pend:
                p, c, o_nat = tp_pend.pop(0)
                for s in range(MQ):
                    tps = pvps.tile([P, P], bf16, tag="pv", name="tps")
                    nc.tensor.transpose(tps, o_nat[:, s, :], ident_sb)
                    nc.vector.tensor_scalar_add(
                        o_sb[:, p, c * TQC + s * P : c * TQC + (s + 1) * P],
                        tps,
                        bvT_sb[:, p : p + 1],
                    )

        def emit_oproj(c):
            tq0 = c * TQC
            for j in range(D // P):
                ys = qkvps.tile([P, TQC], f32, tag="qkv", name="ys")
                for ki in range(MQ):
                    nc.tensor.matmul(
                        ys,
                        wo_sb[:, ki, j * P : (j + 1) * P],
                        o_sb[:, ki, tq0 : tq0 + TQC],
                        start=(ki == 0),
                        stop=(ki == MQ - 1),
                    )
                yo = yop.tile([P, TQC], bf16, name="yo")
                nc.vector.tensor_copy(yo, ys)
                nc.sync.dma_start(out=yT[:, j, tq0 : tq0 + TQC], in_=yo)

        def emit_tail_pv(p, c, pts, tp_pool=None, tp_tag="sc"):
            """s-pipelined PV -> normalize -> transpose chain emitted inside
            its own window (block-3 windows), so the PV matmuls dribble in as
            the exps land and the chunk completes a window earlier than the
            deferred path; the transposes borrow another pool's psum slots to
            avoid a rotation conflict with the still-live opv tiles."""
            tq0 = c * TQC
            o_nat = onp.tile([P, MQ, P], bf16, tag="onat", name="o_nat")
            opvs = [
                pvps.tile([P, MQ, VW], f32, tag="pv", name="opv"),
                pvps.tile([P, MQ, VW], f32, tag="pv", name="opv"),
            ]
            rcs = [
                nrm.tile([P, MQ, 1], f32, name="rc"),
                nrm.tile([P, MQ, 1], f32, name="rc"),
            ]
            for s in range(MQ):
                for i in range(2):
                    h = 2 * p + i
                    for tk in range(TK):
                        nc.tensor.matmul(
                            opvs[i][:, s, :],
                            pts[i][:, tk, s * P : (s + 1) * P],
                            v_sb[:, tk, h * VW : (h + 1) * VW],
                            start=(tk == 0),
                            stop=(tk == TK - 1),
                        )
                    nc.vector.reciprocal(rcs[i][:, s, :], opvs[i][:, s, HD : HD + 1])
                    nc.vector.tensor_scalar_mul(
                        o_nat[:, s, i * HD : (i + 1) * HD],
                        opvs[i][:, s, 0:HD],
                        rcs[i][:, s, :],
                    )
                tps = (tp_pool or scps).tile([P, P], bf16, tag=tp_tag, name="tps")
                nc.tensor.transpose(tps, o_nat[:, s, :], ident_sb)
                nc.vector.tensor_scalar_add(
                    o_sb[:, p, tq0 + s * P : tq0 + (s + 1) * P],
                    tps,
                    bvT_sb[:, p : p + 1],
                )

        def emit_tail_oproj():
            """Final chunk's output projection, s-split so each j consumes
            the 128-query subtiles as they land in o_sb."""
            tq0 = (NC2 - 1) * TQC
            for j in range(D // P):
                # alternate psum pools (pv/sc are both free in the tail;
                # qkv is left to the chunk-2 projection): 4 effective slots
                # so the yo copies pipeline instead of gating every other j
                if j % 2 == 0:
                    ys = pvps.tile([P, TQC], f32, tag="pv", name="ys")
                else:
                    ys = scps.tile([P, TQC], f32, tag="sc", name="ys")
                for s in range(MQ):
                    for ki in range(MQ):
                        nc.tensor.matmul(
                            ys[:, s * P : (s + 1) * P],
                            wo_sb[:, ki, j * P : (j + 1) * P],
                            o_sb[:, ki, tq0 + s * P : tq0 + (s + 1) * P],
                            start=(ki == 0),
                            stop=(ki == MQ - 1),
                        )
                yo = yop.tile([P, TQC], bf16, name="yo")
                if j % 2 == 0:
                    nc.vector.tensor_copy(yo, ys)
                else:
                    nc.scalar.activation(yo, ys, AF.Copy)
                nc.sync.dma_start(out=yT[:, j, tq0 : tq0 + TQC], in_=yo)

        # ---- schedule: pair-outer, chunk-inner ----
        if reps > 1:
            loop_cm = tc.For_i(0, reps, 1)
            loop_cm.__enter__()

        # only the first projections (K n0, Q(0,0)) precede the first scores
        # in emission: everything DMA-gated after that is emitted as window
        # fill so a late x chunk can never head-of-line block ready scores
        emit_qk_part(0, 0, "k")
        emit_qk_part(0, 0, "q")

        for p in range(MQ):
            for c in range(NC2):
                pts = [
                    ptp.tile([P, TK, TQC], bf16, tag="pt", name="pt0"),
                    ptp.tile([P, TK, TQC], bf16, tag="pt", name="pt1"),
                ]
                # max priority: a ready score matmul must always beat fill
                # work in the scheduler, else the exp stream (the pacer)
                # starves behind bulk projection work
                with tc.high_priority():
                    scores_exp_pair(p, c, pts, k_interleave=(p == 0 and c == 0))
                flush_tp()
                flush_pv()
                last2 = p == MQ - 2 and c == NC2 - 1  # window (2,3)
                if p == 0 and c == 0:
                    # two windows ahead: the early windows carry the DMA-gated
                    # projection backlog, so one window of slack is not enough
                    emit_qk_part(0, 1, "q")
                    emit_qk_part(0, 2, "q")
                elif p == 0 and c == 1:
                    emit_qk_part(0, 3, "q")
                if p == 0 and c == 0:
                    for tg in range(4):      # pair 0's V: all needed by (0,1)
                        emit_v_part(0, tg)
                if p < MQ - 1:
                    if not last2:
                        emit_v_part(p + 1, c)  # next pair's V, a block ahead
                    emit_qk_part(p + 1, c, "k" if last2 else "kq")
                if p == MQ - 1 and c == 0:
                    # fill shifted out of the crowded (2,3) window
                    emit_v_part(MQ - 1, NC2 - 1)
                    emit_qk_part(MQ - 1, NC2 - 1, "q")
                if p == MQ - 1 and c >= 1:
                    # shorten the transpose trail to one window so chunk c-1
                    # completes here, then project it as window fill
                    flush_tp()
                    if c == NC2 - 1:
                        # chunk 3's PV chain gates the end: ahead of oproj(2)
                        emit_tail_pv(p, c, pts)
                    emit_oproj(c - 1)
                if p == MQ - 1 and c == NC2 - 2:
                    # (3,2) runs its PV in-window too (transposes on the qkv
                    # slots — sc is busy with the next window's scores), so
                    # chunk 2 completes here and projects as (3,3) fill
                    emit_tail_pv(p, c, pts, tp_pool=qkvps, tp_tag="qkv")
                if not (p == MQ - 1 and c >= NC2 - 2):
                    pv_pend.append((p, c, pts))
        emit_tail_oproj()

        if reps > 1:
            loop_cm.__exit__(None, None, None)

        if reps > 1:
            loop_cm.__exit__(None, None, None)

        if dbg is not None:
            nc.sync.dma_start(out=dbg["q"], in_=q_sb)
            nc.sync.dma_start(out=dbg["k"], in_=k_sb)
            nc.sync.dma_start(out=dbg["v"], in_=v_sb)
            nc.sync.dma_start(out=dbg["o"], in_=o_sb)


def _build(debug=False, reps=1):
    import concourse.tile as tile
    from concourse import bacc, mybir

    dt = mybir.dt
    f32, bf16 = dt.float32, dt.bfloat16

    nc = bacc.Bacc("TRN2", target_bir_lowering=False, debug=False)
    # inputs are host-preswizzled into partition-major layouts so every DMA
    # descriptor is a fat contiguous run
    aps = {
        "xT": nc.dram_tensor("xT", [P, KD, T], bf16, kind="ExternalInput").ap(),
        "wq": nc.dram_tensor("wq", [P, KD, DG], bf16, kind="ExternalInput").ap(),
        "wk": nc.dram_tensor("wk", [P, KD, DG], bf16, kind="ExternalInput").ap(),
        "wv": nc.dram_tensor("wv", [P, KD, DG], bf16, kind="ExternalInput").ap(),
        "wo": nc.dram_tensor("wo", [P, MQ, D], bf16, kind="ExternalInput").ap(),
        "bq": nc.dram_tensor("bq", [P, MQ], f32, kind="ExternalInput").ap(),
        "bk": nc.dram_tensor("bk", [P, MQ], f32, kind="ExternalInput").ap(),
        "bvT": nc.dram_tensor("bvT", [P, MQ], f32, kind="ExternalInput").ap(),
        "ident": nc.dram_tensor("ident", [P, P], bf16, kind="ExternalInput").ap(),
        "yT": nc.dram_tensor("yT", [P, D // P, T], bf16, kind="ExternalOutput").ap(),
    }

    dbg = None
    if debug:
        dbg = {
            "q": nc.dram_tensor("dbg_q", [P, MQ, T], bf16, kind="ExternalOutput").ap(),
            "k": nc.dram_tensor("dbg_k", [P, MQ, T], bf16, kind="ExternalOutput").ap(),
            "v": nc.dram_tensor(
                "dbg_v", [P, TK, HG * VW], bf16, kind="ExternalOutput"
            ).ap(),
            "o": nc.dram_tensor("dbg_o", [P, MQ, T], bf16, kind="ExternalOutput").ap(),
        }

    with tile.TileContext(nc) as tc:
        _emit(tc, aps, dbg, reps=reps)
    nc.compile()
    return nc


def _get_nc():
    if "nc" not in _CACHE:
        _CACHE["nc"] = _build()
    return _CACHE["nc"]


def _shard_inputs(x, Wq, bq, Wk, bk, Wv, bv, Wo, bo):
    import ml_dtypes

    bf16 = ml_dtypes.bfloat16
    f32 = np.float32

    def c(a, dtype):
        return np.ascontiguousarray(a).astype(dtype)

    def kp(a, kt):  # [kt*P, F] -> [P, kt, F] partition-major swizzle
        return a.reshape(kt, P, a.shape[-1]).transpose(1, 0, 2)

    ident = np.eye(P, dtype=bf16)
    in_maps = []
    for core in range(NCORES):
        b, g = core // 2, core % 2
        hs = g * DG
        in_maps.append(
            {
                "xT": c(kp(x[b].T, KD), bf16),
                "wq": c(kp(Wq[hs : hs + DG, :].T, KD), bf16),
                "wk": c(kp(Wk[hs : hs + DG, :].T, KD), bf16),
                "wv": c(kp(Wv[hs : hs + DG, :].T, KD), bf16),
                "wo": c(kp(Wo[:, hs : hs + DG].T, MQ), bf16),
                "bq": c(bq[hs : hs + DG].reshape(MQ, P).T, f32),
                "bk": c(bk[hs : hs + DG].reshape(MQ, P).T, f32),
                "bvT": c(bv[hs : hs + DG].reshape(MQ, P).T, f32),
                "ident": ident,
            }
        )
    return in_maps


def _run(inputs, trace=False):
    from concourse import bass_utils

    nc = _get_nc()
    np_in = {k: np.asarray(v) for k, v in inputs.items()}
    in_maps = _shard_inputs(**np_in)
    res = bass_utils.run_bass_kernel_spmd(
        nc, in_maps, core_ids=list(range(NCORES)), trace=trace
    )
    bo = np_in["bo"].astype(np.float32)
    y = np.empty((B, T, D), dtype=np.float32)
    for b in range(B):
        acc = res.results[2 * b]["yT"].astype(np.float32) + res.results[
            2 * b + 1
        ]["yT"].astype(np.float32)  # [P, D/P, T]
        y[b] = acc.transpose(1, 0, 2).reshape(D, T).T + bo
    return y, res


def kernel(**inputs):
    y, _ = _run(inputs)
    return y
